# revision 7
# baseline (speedup 1.0000x reference)
"""Head-axis-softmax attention on 8 TRN2 NeuronCores.

Problem: q,k,v [4,16,2048,64] f32.
  score = einsum(bhqd,bhkd->bhqk)/8
  attention = softmax(score, axis=1)   # over the 16 HEADS
  z = einsum(bhqk,bhkd->bhqd)
Returns (z, attention).

Sharding: core i -> (b = i//2, q-half = i%2). Softmax couples heads only,
so (b, q) sharding needs no communication. Each core:
  q-shard [16,1024,64], k/v full-k [16,2048,64]
  outputs attention-shard [16,1024,2048] f32 (134 MB) and zT [16,64,1024].

Per-core device plan, per q-block qb of 512 rows:
  pass 1 ([q-part, k-free], blocks of 128 q x 512 k):
    16 scores via fp32r matmul (qT stationary, kT moving) -> PSUM pairs,
    exp(score/8) on ScalarE -> E bf16 [128, 16h*512k]; den = sum_h E
    (DVE bf16 tree, in place); recip = 1/den; PE-transpose recip into
    rTall ([k-part, q-free], for pass 2); A = E*recip in place;
    one SWDGE cast-DMA (bf16->f32) per block -> attention out.
  pass 2 ([k-part, q-free], per head-pair, k in 128-chunks):
    scoresT via fp32r matmul (kT stationary, qT moving), exp -> ET bf16,
    AT = ET*rTall chunk in place; AV: zT[d,q] += v[k,d]^T . AT[k,q]
    accumulated in one PSUM bank per head pair (odd head at output
    base-partition 64); drain -> zT f32 out.

Host packs qT/kT (d-on-partitions, head pairs stacked to 128 partitions)
and v (bf16, k-on-partitions) so the device does zero input transposes.
"""

import numpy as np
import ml_dtypes

B, H, S, D = 4, 16, 2048, 64
SQ = S // 2          # per-core q rows
NCORES = 8
KQ = 512             # pass-1 k block
QB = 512             # pass-2 q block
NKC = S // 128       # 16 k-chunks of 128

_compiled = None


def _build(niter=1):
    import concourse.mybir as mybir
    import concourse.tile as tile
    from concourse import bacc
    from concourse.masks import make_identity

    f32 = mybir.dt.float32
    bf16 = mybir.dt.bfloat16

    nc = bacc.Bacc("TRN2", target_bir_lowering=False, debug=False,
                   num_devices=NCORES)

    qT = nc.dram_tensor("qT", [H // 2, 128, SQ], mybir.dt.float32r, kind="ExternalInput").ap()
    kT = nc.dram_tensor("kT", [H // 2, 128, S], mybir.dt.float32r, kind="ExternalInput").ap()
    vp = nc.dram_tensor("vp", [128, NKC * H * D], bf16,
                        kind="ExternalInput").ap()
    attn = nc.dram_tensor("attn", [H, SQ, S], f32, kind="ExternalOutput").ap()
    zT = nc.dram_tensor("zT", [H, D, SQ], f32, kind="ExternalOutput").ap()

    with tile.TileContext(nc) as tc:
        _body(tc, nc, mybir, make_identity, qT, kT, vp, attn, zT, niter)

    nc.compile()
    return nc


def _body(tc, nc, mybir, make_identity, qT, kT, vp, attn, zT, niter=1):
    from contextlib import ExitStack

    f32 = mybir.dt.float32
    f32r = mybir.dt.float32r
    bf16 = mybir.dt.bfloat16
    EXP = mybir.ActivationFunctionType.Exp

    ctx = ExitStack()
    with ctx:
        ctx.enter_context(nc.allow_low_precision(
            reason="attention probs in [0,1]; bf16 ok vs 2e-2 gate"))
        const = ctx.enter_context(tc.tile_pool(name="const", bufs=1))
        statics = ctx.enter_context(tc.tile_pool(name="statics", bufs=1))
        epool = ctx.enter_context(tc.tile_pool(name="epool", bufs=2))
        t8p = ctx.enter_context(tc.tile_pool(name="t8p", bufs=1))
        small = ctx.enter_context(tc.tile_pool(name="small", bufs=3))
        rtp = ctx.enter_context(tc.tile_pool(name="rtp", bufs=1))
        etp = ctx.enter_context(tc.tile_pool(name="etp", bufs=2))
        zsp = ctx.enter_context(tc.tile_pool(name="zsp", bufs=2))
        psum = ctx.enter_context(tc.tile_pool(name="psum", bufs=3, space="PSUM"))
        psz = ctx.enter_context(tc.tile_pool(name="psz", bufs=2, space="PSUM"))

        ident = const.tile([128, 128], bf16)
        make_identity(nc, ident[:])

        # ---- static inputs -------------------------------------------------
        qTs = statics.tile([128, (H // 2) * SQ], f32r, tag="qTs")
        nc.sync.dma_start(
            qTs[:].rearrange("p (h q) -> p h q", h=H // 2),
            qT[:, :, :].rearrange("h p q -> p h q"))
        kTs = statics.tile([128, (H // 2) * S], f32r, tag="kTs")
        nc.sync.dma_start(
            kTs[:].rearrange("p (h k) -> p h k", h=H // 2),
            kT[:, :, :].rearrange("h p k -> p h k"))
        vps = statics.tile([128, NKC * H * D], bf16, tag="vps")
        nc.sync.dma_start(vps[:], vp[:])

        def qT_ap(h, q0, qn):
            # [64, qn] fp32r slice for head h (pairs packed on partitions)
            return qTs[:].rearrange("p (h q) -> p h q", h=H // 2)[
                (h % 2) * 64:(h % 2) * 64 + 64, h // 2, q0:q0 + qn
            ]

        def kT_ap(h, k0, kn):
            return kTs[:].rearrange("p (h k) -> p h k", h=H // 2)[
                (h % 2) * 64:(h % 2) * 64 + 64, h // 2, k0:k0 + kn
            ]

        def v_ap(h, kkg):
            # [128, 64] bf16: v rows kkg*128..+128 of head h
            return vps[:].rearrange("p (c h d) -> p c h d", c=NKC, h=H)[
                :, kkg, h, :]

        def main_body():
          for qb in range(SQ // QB):
            # rTall: recip transposed, [k-part, q-free] for this q-block:
            # [128, NKC kkg-chunks, QB] bf16
            rTall = rtp.tile([128, NKC * QB], bf16, tag="rT")
            rTv = rTall[:].rearrange("p (c q) -> p c q", c=NKC)

            # ================= PASS 1 =====================================
            for qs in range(qb * (QB // 128), (qb + 1) * (QB // 128)):
                for kq in range(S // KQ):
                    E = epool.tile([128, H * KQ], bf16, tag="E")
                    Ev = E[:].rearrange("p (h k) -> p h k", h=H)
                    for hp in range(H // 2):
                        sc = psum.tile([128, 1024], f32, tag="sc")
                        for j in range(2):
                            h = 2 * hp + j
                            nc.tensor.matmul(
                                sc[:, j * 512:(j + 1) * 512],
                                qT_ap(h, qs * 128, 128),
                                kT_ap(h, kq * KQ, KQ))
                        nc.scalar.activation(
                            E[:, hp * 1024:(hp + 1) * 1024], sc[:], EXP,
                            scale=0.125)
                    # den = sum over heads: batched binary tree
                    # level 1: 8 pair-sums of the 16 heads
                    t8 = t8p.tile([128, 8 * KQ], bf16, tag="t8")
                    pair = E[:].rearrange("p (g two k) -> p g two k", two=2,
                                          k=KQ)
                    t8v = t8[:].rearrange("p (g k) -> p g k", g=8)
                    nc.vector.tensor_add(t8v, pair[:, :, 0, :],
                                         pair[:, :, 1, :])
                    # level 2: 4 sums into t8[0:4] (in place, writes trail reads)
                    t8q = t8[:].rearrange("p (g two k) -> p g two k", two=2,
                                          k=KQ)
                    nc.vector.tensor_add(t8v[:, 0:4, :],
                                         t8q[:, :, 0, :], t8q[:, :, 1, :])
                    # level 3: 2 sums into t8[0:2]
                    t4q = t8v[:, 0:4, :].rearrange("p (a two) k -> p a two k",
                                                   two=2)
                    nc.vector.tensor_add(t8v[:, 0:2, :],
                                         t4q[:, :, 0, :], t4q[:, :, 1, :])
                    den = small.tile([128, KQ], f32, tag="den")
                    nc.vector.tensor_add(den[:], t8v[:, 0, :], t8v[:, 1, :])
                    rec = small.tile([128, KQ], bf16, tag="rec")
                    nc.vector.reciprocal(rec[:], den[:])
                    # transpose recip into rTall (4 kkg chunks)
                    pt = psz.tile([128, 512], bf16, tag="zp")
                    for j in range(4):
                        nc.tensor.transpose(
                            pt[:, j * 128:(j + 1) * 128],
                            rec[:, j * 128:(j + 1) * 128], ident[:])
                    nc.vector.tensor_copy(
                        rTv[:, 4 * kq:4 * kq + 4,
                            (qs % 4) * 128:(qs % 4) * 128 + 128],
                        pt[:].rearrange("p (c q) -> p c q", c=4))
                    # A = E * recip (in place), recip broadcast over h
                    nc.vector.tensor_mul(
                        Ev, Ev,
                        rec[:].unsqueeze(1).broadcast_to([128, H, KQ]))
                    # attention out (bf16 -> f32 cast in SWDGE DMA)
                    nc.gpsimd.dma_start(
                        attn[:, qs * 128:(qs + 1) * 128,
                             kq * KQ:(kq + 1) * KQ].transpose([1, 0, 2]),
                        Ev)

            # ================= PASS 2 =====================================
            for hp in range(H // 2):
                zp = psz.tile([128, 512], f32, tag="zp")
                for kkg in range(NKC):
                    sc = psum.tile([128, 1024], f32, tag="sc")
                    ET = etp.tile([128, 1024], bf16, tag="ET")
                    for j in range(2):
                        h = 2 * hp + j
                        nc.tensor.matmul(
                            sc[:, j * 512:(j + 1) * 512],
                            kT_ap(h, kkg * 128, 128),
                            qT_ap(h, qb * QB, QB))
                    nc.scalar.activation(ET[:], sc[:], EXP, scale=0.125)
                    ETv = ET[:].rearrange("p (h q) -> p h q", h=2)
                    nc.vector.tensor_mul(
                        ETv, ETv,
                        rTv[:, kkg, :].unsqueeze(1).broadcast_to(
                            [128, 2, QB]))
                    for j in range(2):
                        h = 2 * hp + j
                        nc.tensor.matmul(
                            zp[j * 64:(j + 1) * 64, :],
                            v_ap(h, kkg),
                            ET[:, j * 512:(j + 1) * 512],
                            start=(kkg == 0), stop=(kkg == NKC - 1))
                zs = zsp.tile([128, 512], f32, tag="zs")
                nc.vector.tensor_copy(zs[:], zp[:])
                for j in range(2):
                    h = 2 * hp + j
                    nc.sync.dma_start(
                        zT[h, :, qb * QB:(qb + 1) * QB],
                        zs[j * 64:(j + 1) * 64, :])

        if niter == 1:
            main_body()
        else:
            with tc.For_i(0, niter, 1):
                main_body()


def _pack_inputs(query, key, value):
    """Host-side shard + repack. Returns in_maps for 8 cores."""
    bf16 = ml_dtypes.bfloat16
    in_maps = []
    kT_cache = {}
    v_cache = {}
    for core in range(NCORES):
        b, qh = core // 2, core % 2
        qs = query[b, :, qh * SQ:(qh + 1) * SQ, :]          # [16,1024,64]
        qTp = np.ascontiguousarray(
            qs.transpose(0, 2, 1).reshape(H // 2, 128, SQ)).astype(np.float32)
        if b not in kT_cache:
            kT_cache[b] = np.ascontiguousarray(
                key[b].transpose(0, 2, 1).reshape(H // 2, 128, S)
            ).astype(np.float32)
            # v[b]: [16,2048,64] -> [128, (S/128)*H*D] with p = k%128
            v_cache[b] = np.ascontiguousarray(
                value[b].reshape(H, S // 128, 128, D).transpose(2, 1, 0, 3)
                .reshape(128, NKC * H * D)).astype(bf16)
        in_maps.append({"qT": qTp, "kT": kT_cache[b], "vp": v_cache[b]})
    return in_maps


def kernel(query, key, value):
    global _compiled
    from concourse.bass_utils import run_bass_kernel_spmd

    query = np.asarray(query, dtype=np.float32)
    key = np.asarray(key, dtype=np.float32)
    value = np.asarray(value, dtype=np.float32)

    if _compiled is None:
        _compiled = _build()
    nc = _compiled

    in_maps = _pack_inputs(query, key, value)
    res = run_bass_kernel_spmd(nc, in_maps, core_ids=list(range(NCORES)))
    outs = res.results

    z = np.empty((B, H, S, D), dtype=np.float32)
    attention = np.empty((B, H, S, S), dtype=np.float32)
    for core in range(NCORES):
        b, qh = core // 2, core % 2
        attention[b, :, qh * SQ:(qh + 1) * SQ, :] = outs[core]["attn"]
        z[b, :, qh * SQ:(qh + 1) * SQ, :] = outs[core]["zT"].transpose(0, 2, 1)
    return z, attention


# revision 12
# speedup vs baseline: 1.6000x; 1.6000x over previous
"""Head-axis-softmax attention on 8 TRN2 NeuronCores.

Problem: q,k,v [4,16,2048,64] f32.
  score = einsum(bhqd,bhkd->bhqk)/8
  attention = softmax(score, axis=1)   # over the 16 HEADS
  z = einsum(bhqk,bhkd->bhqd)
Returns (z, attention).

Sharding: core i -> (b = i//2, q-half = i%2). Softmax couples heads only,
so (b, q) sharding needs no communication. Each core:
  q-shard [16,1024,64], k/v full-k [16,2048,64]
  outputs attention-shard [16,1024,2048] f32 (134 MB) and zT [16,64,1024].

Per-core device plan, per q-block qb of 512 rows:
  pass 1 ([q-part, k-free], blocks of 128 q x 512 k):
    16 scores via fp32r matmul (qT stationary, kT moving) -> PSUM pairs,
    exp(score/8) on ScalarE -> E bf16 [128, 16h*512k]; den = sum_h E
    (DVE bf16 tree, in place); recip = 1/den; PE-transpose recip into
    rTall ([k-part, q-free], for pass 2); A = E*recip in place;
    one SWDGE cast-DMA (bf16->f32) per block -> attention out.
  pass 2 ([k-part, q-free], per head-pair, k in 128-chunks):
    scoresT via fp32r matmul (kT stationary, qT moving), exp -> ET bf16,
    AT = ET*rTall chunk in place; AV: zT[d,q] += v[k,d]^T . AT[k,q]
    accumulated in one PSUM bank per head pair (odd head at output
    base-partition 64); drain -> zT f32 out.

Host packs qT/kT (d-on-partitions, head pairs stacked to 128 partitions)
and v (bf16, k-on-partitions) so the device does zero input transposes.
"""

import numpy as np
import ml_dtypes

B, H, S, D = 4, 16, 2048, 64
SQ = S // 2          # per-core q rows
NCORES = 8
KQ = 512             # pass-1 k block
QB = 512             # pass-2 q block
NKC = S // 128       # 16 k-chunks of 128

_compiled = None


def _build(niter=1):
    import concourse.mybir as mybir
    import concourse.tile as tile
    from concourse import bacc
    from concourse.masks import make_identity

    f32 = mybir.dt.float32
    bf16 = mybir.dt.bfloat16

    nc = bacc.Bacc("TRN2", target_bir_lowering=False, debug=False,
                   num_devices=NCORES)

    qT = nc.dram_tensor("qT", [H // 2, 128, SQ], mybir.dt.float32r, kind="ExternalInput").ap()
    kT = nc.dram_tensor("kT", [H // 2, 128, S], mybir.dt.float32r, kind="ExternalInput").ap()
    vp = nc.dram_tensor("vp", [128, NKC * H * D], bf16,
                        kind="ExternalInput").ap()
    attn = nc.dram_tensor("attn", [H, SQ, S], f32, kind="ExternalOutput").ap()
    zT = nc.dram_tensor("zT", [H, D, SQ], f32, kind="ExternalOutput").ap()

    with tile.TileContext(nc) as tc:
        _body(tc, nc, mybir, make_identity, qT, kT, vp, attn, zT, niter)

    nc.compile()
    return nc


def _body(tc, nc, mybir, make_identity, qT, kT, vp, attn, zT, niter=1):
    from contextlib import ExitStack

    f32 = mybir.dt.float32
    f32r = mybir.dt.float32r
    bf16 = mybir.dt.bfloat16
    EXP = mybir.ActivationFunctionType.Exp

    ctx = ExitStack()
    with ctx:
        ctx.enter_context(nc.allow_low_precision(
            reason="attention probs in [0,1]; bf16 ok vs 2e-2 gate"))
        const = ctx.enter_context(tc.tile_pool(name="const", bufs=1))
        statics = ctx.enter_context(tc.tile_pool(name="statics", bufs=1))
        epool = ctx.enter_context(tc.tile_pool(name="epool", bufs=2))
        t8p = ctx.enter_context(tc.tile_pool(name="t8p", bufs=1))
        small = ctx.enter_context(tc.tile_pool(name="small", bufs=2))
        rtp = ctx.enter_context(tc.tile_pool(name="rtp", bufs=2))
        etp = ctx.enter_context(tc.tile_pool(name="etp", bufs=2))
        zsp = ctx.enter_context(tc.tile_pool(name="zsp", bufs=1))
        psum = ctx.enter_context(tc.tile_pool(name="psum", bufs=2, space="PSUM"))
        psz = ctx.enter_context(tc.tile_pool(name="psz", bufs=2, space="PSUM"))

        ident = const.tile([128, 128], bf16)
        make_identity(nc, ident[:])

        # ---- static inputs -------------------------------------------------
        qTs = statics.tile([128, (H // 2) * SQ], f32r, tag="qTs")
        nc.sync.dma_start(
            qTs[:].rearrange("p (h q) -> p h q", h=H // 2),
            qT[:, :, :].rearrange("h p q -> p h q"))
        kTs = statics.tile([128, (H // 2) * S], f32r, tag="kTs")
        nc.sync.dma_start(
            kTs[:].rearrange("p (h k) -> p h k", h=H // 2),
            kT[:, :, :].rearrange("h p k -> p h k"))
        vps = statics.tile([128, NKC * H * D], bf16, tag="vps")
        nc.sync.dma_start(vps[:], vp[:])

        def qT_ap(h, q0, qn):
            # [64, qn] fp32r slice for head h (pairs packed on partitions)
            return qTs[:].rearrange("p (h q) -> p h q", h=H // 2)[
                (h % 2) * 64:(h % 2) * 64 + 64, h // 2, q0:q0 + qn
            ]

        def kT_ap(h, k0, kn):
            return kTs[:].rearrange("p (h k) -> p h k", h=H // 2)[
                (h % 2) * 64:(h % 2) * 64 + 64, h // 2, k0:k0 + kn
            ]

        def v_ap(h, kkg):
            # [128, 64] bf16: v rows kkg*128..+128 of head h
            return vps[:].rearrange("p (c h d) -> p c h d", c=NKC, h=H)[
                :, kkg, h, :]

        def pass1_gen(qb, rTv):
            # ================= PASS 1 =====================================
            for qs in range(qb * (QB // 128), (qb + 1) * (QB // 128)):
                for kq in range(S // KQ):
                    E = epool.tile([128, H * KQ], bf16, tag="E")
                    Ev = E[:].rearrange("p (h k) -> p h k", h=H)
                    for hp in range(H // 2):
                        sc = psum.tile([128, 1024], f32, tag="sc")
                        for j in range(2):
                            h = 2 * hp + j
                            nc.tensor.matmul(
                                sc[:, j * 512:(j + 1) * 512],
                                qT_ap(h, qs * 128, 128),
                                kT_ap(h, kq * KQ, KQ))
                        nc.scalar.activation(
                            E[:, hp * 1024:(hp + 1) * 1024], sc[:], EXP,
                            scale=0.125)
                    # den = sum over heads: two 8-head binary trees
                    # sharing one [128, 4*KQ] scratch
                    t4 = t8p.tile([128, 4 * KQ], bf16, tag="t4")
                    t4v = t4[:].rearrange("p (g k) -> p g k", g=4)
                    t4q = t4[:].rearrange("p (g two k) -> p g two k", two=2,
                                          k=KQ)
                    den = small.tile([128, KQ], f32, tag="den")
                    for half in range(2):
                        hv = Ev[:, half * 8:(half + 1) * 8, :].rearrange(
                            "p (g two) k -> p g two k", two=2)
                        nc.vector.tensor_add(t4v, hv[:, :, 0, :],
                                             hv[:, :, 1, :])
                        nc.vector.tensor_add(t4v[:, 0:2, :],
                                             t4q[:, :, 0, :], t4q[:, :, 1, :])
                        if half == 0:
                            nc.vector.tensor_add(den[:], t4v[:, 0, :],
                                                 t4v[:, 1, :])
                        else:
                            nc.vector.tensor_add(t4v[:, 0, :], t4v[:, 0, :],
                                                 t4v[:, 1, :])
                    nc.vector.tensor_add(den[:], den[:], t4v[:, 0, :])
                    # rec32 = 1/den in place (custom-DVE NR approx, ~18 bits)
                    nc.vector.reciprocal_approx_fast(den[:], den[:])
                    rec = small.tile([128, KQ], bf16, tag="rec")
                    nc.vector.tensor_copy(rec[:], den[:])
                    # transpose recip into rTall (4 kkg chunks)
                    pt = psz.tile([128, 512], bf16, tag="pt")
                    for j in range(4):
                        nc.tensor.transpose(
                            pt[:, j * 128:(j + 1) * 128],
                            rec[:, j * 128:(j + 1) * 128], ident[:])
                    nc.vector.tensor_copy(
                        rTv[:, 4 * kq:4 * kq + 4,
                            (qs % 4) * 128:(qs % 4) * 128 + 128],
                        pt[:].rearrange("p (c q) -> p c q", c=4))
                    # A = E * recip (in place), recip broadcast over h
                    nc.vector.tensor_mul(
                        Ev, Ev,
                        rec[:].unsqueeze(1).broadcast_to([128, H, KQ]))
                    # attention out (bf16 -> f32 cast in SWDGE DMA)
                    nc.gpsimd.dma_start(
                        attn[:, qs * 128:(qs + 1) * 128,
                             kq * KQ:(kq + 1) * KQ].transpose([1, 0, 2]),
                        Ev)
                    yield

        def pass2_gen(qb, rTv):
            # ================= PASS 2 =====================================
            for hp in range(H // 2):
                zp = psz.tile([128, 512], f32, tag="zp")
                for kkg in range(NKC):
                    sc = psum.tile([128, 1024], f32, tag="sc")
                    ET = etp.tile([128, 1024], bf16, tag="ET")
                    for j in range(2):
                        h = 2 * hp + j
                        nc.tensor.matmul(
                            sc[:, j * 512:(j + 1) * 512],
                            kT_ap(h, kkg * 128, 128),
                            qT_ap(h, qb * QB, QB))
                    nc.scalar.activation(ET[:], sc[:], EXP, scale=0.125)
                    ETv = ET[:].rearrange("p (h q) -> p h q", h=2)
                    nc.vector.tensor_mul(
                        ETv, ETv,
                        rTv[:, kkg, :].unsqueeze(1).broadcast_to(
                            [128, 2, QB]))
                    for j in range(2):
                        h = 2 * hp + j
                        nc.tensor.matmul(
                            zp[j * 64:(j + 1) * 64, :],
                            v_ap(h, kkg),
                            ET[:, j * 512:(j + 1) * 512],
                            start=(kkg == 0), stop=(kkg == NKC - 1))
                for qh in range(2):
                    zs = zsp.tile([128, 256], f32, tag="zs")
                    nc.vector.tensor_copy(zs[:], zp[:, qh * 256:(qh + 1) * 256])
                    for j in range(2):
                        h = 2 * hp + j
                        nc.sync.dma_start(
                            zT[h, :, qb * QB + qh * 256:qb * QB + (qh + 1) * 256],
                            zs[j * 64:(j + 1) * 64, :])
                yield

        def drive(gen):
            for _ in gen:
                pass

        def new_rtv():
            rTall = rtp.tile([128, NKC * QB], bf16, tag="rT")
            return rTall[:].rearrange("p (c q) -> p c q", c=NKC)

        def main_body():
            nqb = SQ // QB
            rtv = new_rtv()
            drive(pass1_gen(0, rtv))
            for qb in range(nqb):
                # interleave pass2(qb) with pass1(qb+1): 2 pass-1 blocks
                # per pass-2 head-pair chunk
                p2 = pass2_gen(qb, rtv)
                if qb + 1 < nqb:
                    rtv = new_rtv()
                    p1 = pass1_gen(qb + 1, rtv)
                else:
                    p1 = None
                for _ in range(H // 2):
                    next(p2, None)
                    if p1 is not None:
                        next(p1, None)
                        next(p1, None)
                drive(p2)
                if p1 is not None:
                    drive(p1)

        if niter == 1:
            main_body()
        else:
            with tc.For_i(0, niter, 1):
                main_body()


def _pack_inputs(query, key, value):
    """Host-side shard + repack. Returns in_maps for 8 cores."""
    bf16 = ml_dtypes.bfloat16
    in_maps = []
    kT_cache = {}
    v_cache = {}
    for core in range(NCORES):
        b, qh = core // 2, core % 2
        qs = query[b, :, qh * SQ:(qh + 1) * SQ, :]          # [16,1024,64]
        qTp = np.ascontiguousarray(
            qs.transpose(0, 2, 1).reshape(H // 2, 128, SQ)).astype(np.float32)
        if b not in kT_cache:
            kT_cache[b] = np.ascontiguousarray(
                key[b].transpose(0, 2, 1).reshape(H // 2, 128, S)
            ).astype(np.float32)
            # v[b]: [16,2048,64] -> [128, (S/128)*H*D] with p = k%128
            v_cache[b] = np.ascontiguousarray(
                value[b].reshape(H, S // 128, 128, D).transpose(2, 1, 0, 3)
                .reshape(128, NKC * H * D)).astype(bf16)
        in_maps.append({"qT": qTp, "kT": kT_cache[b], "vp": v_cache[b]})
    return in_maps


def kernel(query, key, value):
    global _compiled
    from concourse.bass_utils import run_bass_kernel_spmd

    query = np.asarray(query, dtype=np.float32)
    key = np.asarray(key, dtype=np.float32)
    value = np.asarray(value, dtype=np.float32)

    if _compiled is None:
        _compiled = _build()
    nc = _compiled

    in_maps = _pack_inputs(query, key, value)
    res = run_bass_kernel_spmd(nc, in_maps, core_ids=list(range(NCORES)))
    outs = res.results

    z = np.empty((B, H, S, D), dtype=np.float32)
    attention = np.empty((B, H, S, S), dtype=np.float32)
    for core in range(NCORES):
        b, qh = core // 2, core % 2
        attention[b, :, qh * SQ:(qh + 1) * SQ, :] = outs[core]["attn"]
        z[b, :, qh * SQ:(qh + 1) * SQ, :] = outs[core]["zT"].transpose(0, 2, 1)
    return z, attention
